# revision 42
# baseline (speedup 1.0000x reference)
"""Conv1d (K=5, pad=2) with folded LoRA on 8 Trainium2 NeuronCores.

Strategy
--------
Data-parallel: batch 8 -> 1 batch item per core. LoRA is folded into the
conv weights on the host:
    w_eff = conv_w + (alpha/rank) * einsum('or,rik->oik', lora_B, lora_A)

The device kernel runs entirely in fp8-e4m3 DoubleRow matmuls (0.5
cycles/output-column with a 256-wide contraction -- 4x the per-column fp32r
rate). Precision is recovered with a hi/lo split computed on the host:

    x_hi = e4m3(x)                  x_lo  = e4m3(x - x_hi)
    W16  = e4m3(16*w_eff)           WCOR  = e4m3(16*(w_eff - W16/16))

    psum = W16@x_hi + W16@x_lo + WCOR@x_hi        (WCOR only for the
    y    = psum/16 + bias                          N_CORR highest-energy taps)

Per psum tile [128co, 512t]: 5 hi + 5 lo + N_CORR correction DoubleRow
matmuls, each pairing the two ci-blocks in the DoubleRow slots. Eviction is
one DVE tensor_scalar affine (psum * 1/16 + bias) straight to fp16 staging;
outputs travel as fp16 and are upcast on the host. Measured end-to-end
rel-err vs the fp64 reference: 1.9e-2 @ N_CORR=2 (gate 2e-2), 1.7e-2 @ 3.

Toolchain constraint baked into the structure: every instruction may carry
at most ONE sync wait (walrus setupSyncWait limit), and Tile's wait elision
is per-proc. Hence (same architecture as the fp32r predecessor):
  - PE "observer" matmuls (tiny, scratch PSUM) absorb each x/weight DMA
    lane wait so real matmuls only wait on the DVE sem (PSUM-bank WAR).
    Observers for later-needed tensors (co1 weights, wcor, chunk-0 second
    half) are deferred to just before their first consumer so the stream
    starts as soon as the first ~1.2 MB lands.
  - All input DMAs are issued upfront on the SP HWDGE ring so the stores
    (SWDGE, one fresh DMASW lane each, at most 8) queue behind them on the
    shared DMA engines and never delay a load the PE is about to need.
  - Evictions run exclusively on DVE and wait only on PE; the fp16 staging
    tile covers the full output (no reuse -> no WAR gates); the bias lane
    is absorbed by a tiny DVE copy. The final store covers only the last
    512 columns so the post-stream tail is short.
  - A tail chain of 1-dep sync nops covers all procs so the exit drain
    carries at most one wait.

TimelineSim (the graded cost model): 92431 ns vs 177471 ns for the fp32r
predecessor. Steady-state marginal cost per 1024-column chunk is 5138 ns
vs the 5120 ns PE ideal; the remaining ~10.5 us is fixed startup (DMA
pipeline lead + first transfers, ~3.2 us), eviction+store tail (~2.6 us),
and Tile's exit drain/barrier sequence (~2.5 us).
"""
import sys
sys.path.insert(0, "/opt/trn_rl_repo")
import numpy as np
import ml_dtypes

from concourse import bass, mybir, tile
from concourse import bass_utils
from concourse.tile import add_dep_helper

E4M3 = ml_dtypes.float8_e4m3fn

# Problem constants (hardcoded per contract)
B = 8
CI = 256
CO = 256
K = 5
PAD = 2
T = 16384
RANK = 8
ALPHA = 16.0
SCALING = ALPHA / RANK
N_CORES = 8

N_CORR = 2            # correction taps (w-error fix); 2 -> rel_err ~1.9e-2

# Tiling
CHUNK = 1024          # output columns per chunk
NCHUNK = T // CHUNK   # 16
SUB = 512             # matmul free dim (one PSUM bank)
NSUB = CHUNK // SUB   # 2
XCOLS = CHUNK + 2 * PAD  # chunk + halo


def _build_nc(corr_taps, _probe_no_evict=False, _probe_no_store=False):
    f32 = mybir.dt.float32
    f16 = mybir.dt.float16
    f8 = mybir.dt.float8e4
    DR = mybir.MatmulPerfMode.DoubleRow
    n_corr = len(corr_taps)

    nc = bass.Bass(trn_type="TRN2", debug=False)
    # x slots: 0,1 = x_hi(ci blk 0/1); 2,3 = x_lo(ci blk 0/1)
    x = nc.dram_tensor("x", [128, 4, T], f8, kind="ExternalInput").ap()
    wts = nc.dram_tensor("wts", [128, K * 2 * 2 * 128], f8,
                         kind="ExternalInput").ap()
    bias = nc.dram_tensor("bias", [128, 2], f32, kind="ExternalInput").ap()
    zeros = nc.dram_tensor("zeros", [128, 4, PAD], f8, kind="ExternalInput").ap()
    if n_corr:
        wcor = nc.dram_tensor("wcor", [128, n_corr * 2 * 2 * 128], f8,
                              kind="ExternalInput").ap()
    # eight output tensors (one per SWDGE store, fresh DMASW lane each); the
    # last covers only the final 512 columns so the tail transfer is short.
    # Host concatenates along columns.
    Y_COLS = [4 * CHUNK] + [2 * CHUNK] * 5 + [CHUNK + SUB, SUB]
    ys = [nc.dram_tensor(f"y{s}", [128, 2, w], f16, kind="ExternalOutput").ap()
          for s, w in enumerate(Y_COLS)]

    NPB = 6   # psum accumulation banks
    NWARM = 0   # PE warmup matmuls: no-op under TimelineSim's wall-clock
                # p-state model; kept as a knob for real-HW experiments

    with tile.TileContext(nc) as tc:
        with tc.tile_pool(name="wp", bufs=1) as wp, \
             tc.tile_pool(name="pp", bufs=1, space="PSUM") as pp:

            # write-once observer scratch: four columns per observer matmul
            obs_ps = pp.tile([128, 96], f32, name="obs_ps", tag="obs")
            pbufs = [pp.tile([128, SUB], f32, name=f"pt{j}", tag=f"pt{j}")
                     for j in range(NPB)]
            # x is fully resident: one dedicated buffer per chunk, no reuse
            xbufs = [wp.tile([128, 4, XCOLS], f8, name=f"xt{j}", tag=f"xt{j}")
                     for j in range(NCHUNK)]
            # single full-width staging tile: stores slice arbitrary ranges
            ot_all = wp.tile([128, 2, T], f16, name="ot_all")

            if NWARM:
                # PE warmup: junk tile filled by DVE at t0; matmuls on it ramp
                # the PE p-state while the input DMAs stream in.
                junk = wp.tile([128, 2, 256], f8, name="junk")
                wu_ms = nc.vector.memset(junk[:], 0.0)
                for wi in range(NWARM):
                    wm = nc.tensor.matmul(
                        pbufs[0][:, 0:256],
                        junk[:, :, 0:128], junk[:, :, 0:256],
                        start=True, stop=True, perf_mode=DR)
                    if wi == 0:
                        add_dep_helper(wm.ins, wu_ms.ins, sync=True,
                                       reason="warmup")

            wt = wp.tile([128, 2, K, 2, 128], f8, name="wt")
            wview = wts[:].rearrange("p (c k i m) -> p c k i m", c=2, k=K, i=2)
            bs = wp.tile([128, 2], f32, name="bs")

            n_obs = [0]

            def pe_observe(src_ap, dma_inst):
                """Tiny matmul whose only wait is `dma_inst`'s lane.

                Reads only within the region `dma_inst` wrote; writes its own
                never-reused obs_ps columns (no WAW chain)."""
                n = src_ap.shape[-1]
                m = min(4, n)
                oc = 4 * n_obs[0]
                n_obs[0] += 1
                mm = nc.tensor.matmul(obs_ps[0:m, oc:oc + m], src_ap[:, 0:m],
                                      src_ap[:, 0:m], start=True, stop=True)
                add_dep_helper(mm.ins, dma_inst.ins, sync=False,
                               reason="obs-order")
                return mm

            # --- all input DMAs issued upfront (SP HWDGE ring) so stores
            # queue behind them on the shared DMA engines and never delay a
            # load the PE is about to need. Ordered so the first matmul
            # group's deps (co0 weights + chunk-0 first half) land first;
            # later-needed tensors (co1 weights, wcor, bias, chunk-0 second
            # half) follow, each observed just before its first consumer.
            in_dmas = [[] for _ in range(NCHUNK)]
            # cols of chunk 0 needed by its first (ts=0) groups; chosen so
            # both DMA halves have >= 512-byte runs (single-rate DMA)
            HALF0 = SUB + PAD + PAD
            d_w0 = nc.sync.dma_start(out=wt[:, 0], in_=wview[:, 0])
            in_dmas[0].append(nc.sync.dma_start(
                out=xbufs[0][:, :, PAD:HALF0],
                in_=x[:, :, 0:HALF0 - PAD]))
            in_dmas[0].append(nc.sync.dma_start(
                out=xbufs[0][:, :, 0:PAD], in_=zeros[:]))
            d_w1 = nc.sync.dma_start(out=wt[:, 1], in_=wview[:, 1])
            if n_corr:
                wc = wp.tile([128, n_corr, 2, 2, 128], f8, name="wc")
                d_wc = nc.sync.dma_start(
                    out=wc[:],
                    in_=wcor[:].rearrange("p (j c i m) -> p j c i m",
                                          j=n_corr, c=2, i=2))
            d_b = nc.sync.dma_start(out=bs[:], in_=bias[:])
            in_dmas[0].append(nc.sync.dma_start(
                out=xbufs[0][:, :, HALF0:XCOLS],
                in_=x[:, :, HALF0 - PAD:CHUNK + PAD]))
            for c in range(1, NCHUNK):
                lo = c * CHUNK - PAD
                if c == NCHUNK - 1:
                    in_dmas[c].append(nc.sync.dma_start(
                        out=xbufs[c][:, :, 0:CHUNK + PAD],
                        in_=x[:, :, lo:T]))
                    in_dmas[c].append(nc.sync.dma_start(
                        out=xbufs[c][:, :, CHUNK + PAD:XCOLS],
                        in_=zeros[:]))
                else:
                    in_dmas[c].append(nc.sync.dma_start(
                        out=xbufs[c][:], in_=x[:, :, lo:lo + XCOLS]))

            obs_w0 = pe_observe(wt[:, 0, 0, 0], d_w0)
            # deferred observers, emitted just before their first consumer
            pend_w1 = [d_w1]
            pend_wc = [d_wc] if n_corr else []
            # DVE observes the bias lane via a write-once copy
            bscratch = wp.tile([128, 2], f32, name="bscratch")
            obs_b = nc.vector.tensor_copy(bscratch[:], bs[:])

            out_dmas = []     # store DMAs
            last_mm = None
            last_evict = None
            pi = 0            # psum bank rotation

            def emit_store(s):
                # SWDGE store (fresh DMASW lane each): carries only its
                # staging-ready (DVE evict) wait.
                col0 = sum(Y_COLS[:s])
                out_dmas.append(nc.gpsimd.dma_start(
                    out=ys[s][:], in_=ot_all[:, :, col0:col0 + Y_COLS[s]]))

            first_evict = [True]

            def do_group(c, ts, co, seg):
                """One psum accumulation group + eviction. seg: (lo, n) cols
                within the subtile (for the split tail group)."""
                nonlocal last_mm, last_evict, pi
                slo, sn = seg
                xt = xbufs[c]
                pt = pbufs[pi % NPB]
                pi += 1
                mm_specs = (
                    [(wt[:, co, k], 0, k) for k in range(K)] +
                    [(wt[:, co, k], 2, k) for k in range(K)] +
                    [(wc[:, j, co], 0, k) for j, k in enumerate(corr_taps)])
                for n_i, (st, sl, k) in enumerate(mm_specs):
                    if n_i == 10 and pend_wc:
                        observers.append(pe_observe(wc[:, 0, 0, 0],
                                                    pend_wc.pop()))
                    off = ts * SUB + slo + k
                    mm = nc.tensor.matmul(
                        pt[:, 0:sn],
                        st,
                        xt[:, sl:sl + 2, off:off + sn],
                        start=(n_i == 0),
                        stop=(n_i == len(mm_specs) - 1),
                        perf_mode=DR,
                    )
                    while observers:
                        add_dep_helper(mm.ins, observers.pop().ins,
                                       sync=False, reason="order-after-obs")
                    last_mm = mm
                if _probe_no_evict:
                    return
                off = c * CHUNK + ts * SUB + slo
                ev = nc.vector.tensor_scalar(
                    out=ot_all[:, co, off:off + sn],
                    in0=pt[:, 0:sn],
                    scalar1=1.0 / 16.0,
                    scalar2=bs[:, co:co + 1],
                    op0=mybir.AluOpType.mult,
                    op1=mybir.AluOpType.add,
                )
                if first_evict[0]:
                    add_dep_helper(ev.ins, obs_b.ins, sync=False,
                                   reason="order-after-gate")
                    first_evict[0] = False
                last_evict = ev

            observers = []
            for c in range(NCHUNK):
                xt = xbufs[c]

                # PE observes this chunk's x lanes; each observer reads only
                # within its DMA's region. Chunk 0's second-half observer is
                # deferred until its ts=1 groups so ts=0 can start early.
                late_x = []
                for i, d in enumerate(in_dmas[c]):
                    if c == 0:
                        src_ap = [xt[:, 0, PAD:PAD + 4], xt[:, 0, 0:PAD],
                                  xt[:, 0, HALF0:HALF0 + 4]][i]
                    elif c == NCHUNK - 1:
                        src_ap = [xt[:, 0, 0:4],
                                  xt[:, 0, CHUNK + PAD:XCOLS]][i]
                    else:
                        src_ap = xt[:, 0, 0:4]
                    if c == 0 and i == 2:
                        late_x.append((src_ap, d))
                    else:
                        observers.append(pe_observe(src_ap, d))
                if c == 0:
                    observers.append(obs_w0)

                for ts in range(NSUB):
                    if late_x and ts == 1:
                        observers.extend(pe_observe(s, d) for s, d in late_x)
                        late_x = []
                    for co in range(2):
                        if pend_w1 and co == 1:
                            observers.append(pe_observe(wt[:, 1, 0, 0],
                                                        pend_w1.pop()))
                        do_group(c, ts, co, (0, SUB))
                    if c == NCHUNK - 1 and ts == 0 and not _probe_no_evict \
                            and not _probe_no_store:
                        emit_store(6)   # chunk 14 + first half of chunk 15
                if _probe_no_evict or _probe_no_store:
                    continue
                if c in (3, 5, 7, 9, 11, 13):
                    emit_store((3, 5, 7, 9, 11, 13).index(c))

            if not _probe_no_evict and not _probe_no_store:
                emit_store(7)           # final 512 columns

            # Tail flush: cover every proc with 1-dep sync nops so the final
            # drain carries at most one wait.
            tail_deps = [d for ds in in_dmas[-8:] for d in ds] + out_dmas + \
                [last_mm, last_evict]
            for dep in tail_deps:
                if dep is None:
                    continue
                nop = nc.sync.nop()
                add_dep_helper(nop.ins, dep.ins, sync=True, reason="tailflush")

    return nc


def check_waits(nc):
    """Return instructions carrying more than one sync wait (walrus limit)."""
    bad = []
    for f in nc.m.functions:
        for bb in f.blocks:
            for inst in bb.instructions:
                si = inst.sync_info
                nw = len(si.on_wait) if si and si.on_wait else 0
                if nw > 1:
                    bad.append((inst.name, type(inst).__name__, nw,
                                [w.ant_name for w in si.on_wait]))
    return bad


def _q8(a):
    return np.asarray(a, dtype=np.float32).astype(E4M3)


def _pack_weights(conv_w, conv_b, lora_A, lora_B):
    w_eff = (conv_w.astype(np.float64) + SCALING * np.einsum(
        "or,rik->oik", lora_B.astype(np.float64),
        lora_A.astype(np.float64).reshape(RANK, CI, K))).astype(np.float32)
    W16 = _q8(16.0 * w_eff)
    w_lo = w_eff - W16.astype(np.float32) / 16.0
    energies = [(float(np.square(w_lo[:, :, k]).sum()), k) for k in range(K)]
    corr_taps = tuple(sorted(k for _, k in
                             sorted(energies, reverse=True)[:N_CORR]))
    WCOR = _q8(16.0 * w_lo)

    def pack(w8, taps, order):
        a = w8.astype(np.float32).reshape(2, 128, 2, 128, K)  # [c, m, i, p, k]
        a = a[:, :, :, :, list(taps)]                          # [c, m, i, p, j]
        a = a.transpose(order)
        return np.ascontiguousarray(
            a.reshape(128, len(taps) * 2 * 2 * 128)).astype(E4M3)

    # wts[p, c, k, i, m]; wcor[p, j, c, i, m]
    wts = pack(W16, range(K), (3, 0, 4, 2, 1))
    wcor = pack(WCOR, corr_taps, (3, 4, 0, 2, 1)) if N_CORR else None
    bias = np.ascontiguousarray(
        conv_b.astype(np.float32).reshape(2, 128).T)  # [128, 2]
    return wts, wcor, bias, corr_taps


_CACHED = {}


def kernel(x, conv_w, conv_b, lora_A, lora_B, _trace=False):
    x = np.asarray(x, dtype=np.float32)
    wts, wcor, bias, corr_taps = _pack_weights(
        np.asarray(conv_w), np.asarray(conv_b),
        np.asarray(lora_A), np.asarray(lora_B))
    zeros = np.zeros((128, 4, PAD), dtype=E4M3)

    if corr_taps not in _CACHED:
        nc = _build_nc(corr_taps)
        bad = check_waits(nc)
        assert not bad, f"sync-wait violations: {bad[:5]}"
        _CACHED[corr_taps] = nc
    nc = _CACHED[corr_taps]
    # test.py compatibility handle
    kernel.__globals__["_CACHED_NC"] = nc

    x_hi = _q8(x)
    x_lo = _q8(x - x_hi.astype(np.float32))
    # xpack[core][p, s, t]; s = hl*2 + ci_blk
    xp = np.stack([x_hi.reshape(B, 2, 128, T), x_lo.reshape(B, 2, 128, T)],
                  axis=1)                       # [B, hl, blk, p, t]
    xp = np.ascontiguousarray(xp.transpose(0, 3, 1, 2, 4)  # [B, p, hl, blk, t]
                              .reshape(B, 128, 4, T))

    in_maps = []
    for i in range(N_CORES):
        m = {"x": xp[i], "wts": wts, "bias": bias, "zeros": zeros}
        if wcor is not None:
            m["wcor"] = wcor
        in_maps.append(m)
    res = bass_utils.run_bass_kernel_spmd(
        nc, in_maps, core_ids=list(range(N_CORES)), trace=_trace)
    outs = []
    for i in range(N_CORES):
        yc = np.concatenate([np.asarray(res.results[i][f"y{s}"])
                             for s in range(NCHUNK // 2)], axis=2)
        outs.append(yc.transpose(1, 0, 2).reshape(CO, T))
    out = np.stack(outs, axis=0).astype(np.float32)
    if _trace:
        kernel._last_exec_time_ns = res.exec_time_ns
        kernel._last_results = res
    return out


_CACHED_NC = None


if __name__ == "__main__":
    nc = _build_nc((0, 2))
    bad = check_waits(nc)
    print("violations:", bad[:10])
    n_inst = sum(len(bb.instructions) for f in nc.m.functions for bb in f.blocks)
    print("instructions:", n_inst)
    from concourse.timeline_sim import TimelineSim
    dur = TimelineSim(nc, trace=False).simulate()
    print(f"TimelineSim: {dur:.0f} ns")


# revision 56
# speedup vs baseline: 1.0071x; 1.0071x over previous
"""Conv1d (K=5, pad=2) with folded LoRA on 8 Trainium2 NeuronCores.

Strategy
--------
Data-parallel: batch 8 -> 1 batch item per core. LoRA is folded into the
conv weights on the host:
    w_eff = conv_w + (alpha/rank) * einsum('or,rik->oik', lora_B, lora_A)

The device kernel runs entirely in fp8-e4m3 DoubleRow matmuls (0.5
cycles/output-column with a 256-wide contraction -- 4x the per-column fp32r
rate). Precision is recovered with a hi/lo split computed on the host:

    x_hi = e4m3(x)                  x_lo  = e4m3(x - x_hi)
    W16  = e4m3(16*w_eff)           WCOR  = e4m3(16*(w_eff - W16/16))

    psum = W16@x_hi + W16@x_lo + WCOR@x_hi        (WCOR only for the
    y    = psum/16 + bias                          N_CORR highest-energy taps)

Per psum tile [128co, 512t]: 5 hi + 5 lo + N_CORR correction DoubleRow
matmuls, each pairing the two ci-blocks in the DoubleRow slots. Eviction is
one DVE tensor_scalar affine (psum * 1/16 + bias) straight to fp16 staging;
outputs travel as fp16 and are upcast on the host. Measured end-to-end
rel-err vs the fp64 reference: 1.9e-2 @ N_CORR=2 (gate 2e-2), 1.7e-2 @ 3.

Toolchain constraint baked into the structure: every instruction may carry
at most ONE sync wait (walrus setupSyncWait limit), and Tile's wait elision
is per-proc. Hence (same architecture as the fp32r predecessor):
  - PE "observer" matmuls (tiny, scratch PSUM) absorb each x/weight DMA
    lane wait so real matmuls only wait on the DVE sem (PSUM-bank WAR).
    Observers for later-needed tensors (co1 weights, wcor, chunk-0 second
    half) are deferred to just before their first consumer so the stream
    starts as soon as the first ~1.2 MB lands.
  - All input DMAs are issued upfront on the SP HWDGE ring so the stores
    (SWDGE, one fresh DMASW lane each, at most 8) queue behind them on the
    shared DMA engines and never delay a load the PE is about to need.
  - Evictions run exclusively on DVE and wait only on PE; the fp16 staging
    tile covers the full output (no reuse -> no WAR gates); the bias lane
    is absorbed by a tiny DVE copy. The final store covers only the last
    512 columns so the post-stream tail is short.
  - A tail chain of 1-dep sync nops covers all procs so the exit drain
    carries at most one wait.

TimelineSim (the graded cost model): 91781 ns vs 177471 ns for the fp32r
predecessor. Steady-state marginal cost per 1024-column chunk is 5138 ns
vs the 5120 ns PE ideal; the remaining ~9.9 us is fixed startup (DMA
pipeline lead + first transfers, ~2.7 us), eviction+store tail (~2.6 us),
and Tile's exit drain/barrier sequence (~2.5 us).
"""
import sys
sys.path.insert(0, "/opt/trn_rl_repo")
import numpy as np
import ml_dtypes

from concourse import bass, mybir, tile
from concourse import bass_utils
from concourse.tile import add_dep_helper

E4M3 = ml_dtypes.float8_e4m3fn

# Problem constants (hardcoded per contract)
B = 8
CI = 256
CO = 256
K = 5
PAD = 2
T = 16384
RANK = 8
ALPHA = 16.0
SCALING = ALPHA / RANK
N_CORES = 8

N_CORR = 2            # correction taps (w-error fix); 2 -> rel_err ~1.9e-2

# Tiling
CHUNK = 1024          # output columns per chunk
NCHUNK = T // CHUNK   # 16
SUB = 512             # matmul free dim (one PSUM bank)
NSUB = CHUNK // SUB   # 2
XCOLS = CHUNK + 2 * PAD  # chunk + halo


def _build_nc(corr_taps, _probe_no_evict=False, _probe_no_store=False):
    f32 = mybir.dt.float32
    f16 = mybir.dt.float16
    f8 = mybir.dt.float8e4
    DR = mybir.MatmulPerfMode.DoubleRow
    n_corr = len(corr_taps)

    nc = bass.Bass(trn_type="TRN2", debug=False)
    # x slots: 0,1 = x_hi(ci blk 0/1); 2,3 = x_lo(ci blk 0/1)
    x = nc.dram_tensor("x", [128, 4, T], f8, kind="ExternalInput").ap()
    wts = nc.dram_tensor("wts", [128, K * 2 * 2 * 128], f8,
                         kind="ExternalInput").ap()
    bias = nc.dram_tensor("bias", [128, 2], f32, kind="ExternalInput").ap()
    zeros = nc.dram_tensor("zeros", [128, 4, PAD], f8, kind="ExternalInput").ap()
    if n_corr:
        wcor = nc.dram_tensor("wcor", [128, n_corr * 2 * 2 * 128], f8,
                              kind="ExternalInput").ap()
    # eight output tensors (one per SWDGE store, fresh DMASW lane each); the
    # last covers only the final 512 columns so the tail transfer is short.
    # Host concatenates along columns.
    Y_COLS = [4 * CHUNK] + [2 * CHUNK] * 5 + [CHUNK + SUB, SUB]
    ys = [nc.dram_tensor(f"y{s}", [128, 2, w], f16, kind="ExternalOutput").ap()
          for s, w in enumerate(Y_COLS)]

    NPB = 6   # psum accumulation banks
    NWARM = 0   # PE warmup matmuls: no-op under TimelineSim's wall-clock
                # p-state model; kept as a knob for real-HW experiments

    with tile.TileContext(nc) as tc:
        with tc.tile_pool(name="wp", bufs=1) as wp, \
             tc.tile_pool(name="pp", bufs=1, space="PSUM") as pp:

            # write-once observer scratch: four columns per observer matmul
            obs_ps = pp.tile([128, 96], f32, name="obs_ps", tag="obs")
            pbufs = [pp.tile([128, SUB], f32, name=f"pt{j}", tag=f"pt{j}")
                     for j in range(NPB)]
            # x is fully resident: one dedicated buffer per chunk, no reuse
            xbufs = [wp.tile([128, 4, XCOLS], f8, name=f"xt{j}", tag=f"xt{j}")
                     for j in range(NCHUNK)]
            # single full-width staging tile: stores slice arbitrary ranges
            ot_all = wp.tile([128, 2, T], f16, name="ot_all")

            if NWARM:
                # PE warmup: junk tile filled by DVE at t0; matmuls on it ramp
                # the PE p-state while the input DMAs stream in.
                junk = wp.tile([128, 2, 256], f8, name="junk")
                wu_ms = nc.vector.memset(junk[:], 0.0)
                for wi in range(NWARM):
                    wm = nc.tensor.matmul(
                        pbufs[0][:, 0:256],
                        junk[:, :, 0:128], junk[:, :, 0:256],
                        start=True, stop=True, perf_mode=DR)
                    if wi == 0:
                        add_dep_helper(wm.ins, wu_ms.ins, sync=True,
                                       reason="warmup")

            wt = wp.tile([128, 2, K, 2, 128], f8, name="wt")
            wview = wts[:].rearrange("p (c k i m) -> p c k i m", c=2, k=K, i=2)
            bs = wp.tile([128, 2], f32, name="bs")

            n_obs = [0]

            def pe_observe(src_ap, dma_inst):
                """Tiny matmul whose only wait is `dma_inst`'s lane.

                Reads only within the region `dma_inst` wrote; writes its own
                never-reused obs_ps columns (no WAW chain)."""
                n = src_ap.shape[-1]
                m = min(4, n)
                oc = 4 * n_obs[0]
                n_obs[0] += 1
                mm = nc.tensor.matmul(obs_ps[0:m, oc:oc + m], src_ap[:, 0:m],
                                      src_ap[:, 0:m], start=True, stop=True)
                add_dep_helper(mm.ins, dma_inst.ins, sync=False,
                               reason="obs-order")
                return mm

            # --- all input DMAs issued upfront (SP HWDGE ring) so stores
            # queue behind them on the shared DMA engines and never delay a
            # load the PE is about to need. Ordered so the first matmul
            # group's deps (co0 weights + chunk-0 first half) land first;
            # later-needed tensors (co1 weights, wcor, bias, chunk-0 second
            # half) follow, each observed just before its first consumer.
            in_dmas = [[] for _ in range(NCHUNK)]
            # cols of chunk 0 needed by its first (ts=0) groups; chosen so
            # both DMA halves have >= 512-byte runs (single-rate DMA)
            HALF0 = SUB + PAD + PAD
            # chunk-0 head zeros via DVE memset: skips the serialized DMA
            # device so the first matmul group starts as soon as the co0
            # weights + chunk-0 first half land (~2.7us)
            z0_ms = nc.vector.memset(xbufs[0][:, :, 0:PAD], 0.0)
            d_w0 = nc.sync.dma_start(out=wt[:, 0], in_=wview[:, 0])
            in_dmas[0].append(nc.sync.dma_start(
                out=xbufs[0][:, :, PAD:HALF0],
                in_=x[:, :, 0:HALF0 - PAD]))
            d_w1 = nc.sync.dma_start(out=wt[:, 1], in_=wview[:, 1])
            if n_corr:
                wc = wp.tile([128, n_corr, 2, 2, 128], f8, name="wc")
                d_wc = nc.sync.dma_start(
                    out=wc[:],
                    in_=wcor[:].rearrange("p (j c i m) -> p j c i m",
                                          j=n_corr, c=2, i=2))
            d_b = nc.sync.dma_start(out=bs[:], in_=bias[:])
            in_dmas[0].append(nc.sync.dma_start(
                out=xbufs[0][:, :, HALF0:XCOLS],
                in_=x[:, :, HALF0 - PAD:CHUNK + PAD]))
            for c in range(1, NCHUNK):
                lo = c * CHUNK - PAD
                if c == NCHUNK - 1:
                    in_dmas[c].append(nc.sync.dma_start(
                        out=xbufs[c][:, :, 0:CHUNK + PAD],
                        in_=x[:, :, lo:T]))
                    in_dmas[c].append(nc.sync.dma_start(
                        out=xbufs[c][:, :, CHUNK + PAD:XCOLS],
                        in_=zeros[:]))
                else:
                    in_dmas[c].append(nc.sync.dma_start(
                        out=xbufs[c][:], in_=x[:, :, lo:lo + XCOLS]))

            obs_w0 = pe_observe(wt[:, 0, 0, 0], d_w0)
            # deferred observers, emitted just before their first consumer
            pend_w1 = [d_w1]
            pend_wc = [d_wc] if n_corr else []
            # DVE observes the bias lane via a write-once copy
            bscratch = wp.tile([128, 2], f32, name="bscratch")
            obs_b = nc.vector.tensor_copy(bscratch[:], bs[:])

            out_dmas = []     # store DMAs
            last_mm = None
            last_evict = None
            pi = 0            # psum bank rotation

            def emit_store(s):
                # SWDGE store (fresh DMASW lane each): carries only its
                # staging-ready (DVE evict) wait.
                col0 = sum(Y_COLS[:s])
                out_dmas.append(nc.gpsimd.dma_start(
                    out=ys[s][:], in_=ot_all[:, :, col0:col0 + Y_COLS[s]]))

            first_evict = [True]

            def do_group(c, ts, co, seg, advance_bank=True):
                """One psum accumulation group + eviction. seg: (lo, n) cols
                within the subtile (for the split tail group, which shares one
                bank across its two halves via the slo offset)."""
                nonlocal last_mm, last_evict, pi
                slo, sn = seg
                xt = xbufs[c]
                pt = pbufs[pi % NPB]
                if advance_bank:
                    pi += 1
                mm_specs = (
                    [(wt[:, co, k], 0, k) for k in range(K)] +
                    [(wt[:, co, k], 2, k) for k in range(K)] +
                    [(wc[:, j, co], 0, k) for j, k in enumerate(corr_taps)])
                for n_i, (st, sl, k) in enumerate(mm_specs):
                    if n_i == 10 and pend_wc:
                        observers.append(pe_observe(wc[:, 0, 0, 0],
                                                    pend_wc.pop()))
                    off = ts * SUB + slo + k
                    mm = nc.tensor.matmul(
                        pt[:, slo:slo + sn],
                        st,
                        xt[:, sl:sl + 2, off:off + sn],
                        start=(n_i == 0),
                        stop=(n_i == len(mm_specs) - 1),
                        perf_mode=DR,
                    )
                    while observers:
                        add_dep_helper(mm.ins, observers.pop().ins,
                                       sync=False, reason="order-after-obs")
                    last_mm = mm
                if _probe_no_evict:
                    return
                off = c * CHUNK + ts * SUB + slo
                ev = nc.vector.tensor_scalar(
                    out=ot_all[:, co, off:off + sn],
                    in0=pt[:, slo:slo + sn],
                    scalar1=1.0 / 16.0,
                    scalar2=bs[:, co:co + 1],
                    op0=mybir.AluOpType.mult,
                    op1=mybir.AluOpType.add,
                )
                if first_evict[0]:
                    add_dep_helper(ev.ins, obs_b.ins, sync=False,
                                   reason="order-after-gate")
                    first_evict[0] = False
                last_evict = ev

            observers = []
            for c in range(NCHUNK):
                xt = xbufs[c]

                # PE observes this chunk's x lanes; each observer reads only
                # within its DMA's region. Chunk 0's second-half observer is
                # deferred until its ts=1 groups so ts=0 can start early.
                late_x = []
                for i, d in enumerate(in_dmas[c]):
                    if c == 0:
                        src_ap = [xt[:, 0, PAD:PAD + 4],
                                  xt[:, 0, HALF0:HALF0 + 4]][i]
                    elif c == NCHUNK - 1:
                        src_ap = [xt[:, 0, 0:4],
                                  xt[:, 0, CHUNK + PAD:XCOLS]][i]
                    else:
                        src_ap = xt[:, 0, 0:4]
                    if c == 0 and i == 1:
                        late_x.append((src_ap, d))
                    else:
                        observers.append(pe_observe(src_ap, d))
                if c == 0:
                    observers.append(obs_w0)

                for ts in range(NSUB):
                    if late_x and ts == 1:
                        observers.extend(pe_observe(s, d) for s, d in late_x)
                        late_x = []
                    for co in range(2):
                        if pend_w1 and co == 1:
                            observers.append(pe_observe(wt[:, 1, 0, 0],
                                                        pend_w1.pop()))
                        do_group(c, ts, co, (0, SUB))
                    if c == NCHUNK - 1 and ts == 0 and not _probe_no_evict \
                            and not _probe_no_store:
                        emit_store(6)   # chunk 14 + first half of chunk 15
                if _probe_no_evict or _probe_no_store:
                    continue
                if c in (3, 5, 7, 9, 11, 13):
                    emit_store((3, 5, 7, 9, 11, 13).index(c))

            if not _probe_no_evict and not _probe_no_store:
                emit_store(7)           # final 512 columns

            # Tail flush: cover every proc with 1-dep sync nops so the final
            # drain carries at most one wait.
            tail_deps = [d for ds in in_dmas[-8:] for d in ds] + out_dmas + \
                [last_mm, last_evict]
            for dep in tail_deps:
                if dep is None:
                    continue
                nop = nc.sync.nop()
                add_dep_helper(nop.ins, dep.ins, sync=True, reason="tailflush")

    return nc


def check_waits(nc):
    """Return instructions carrying more than one sync wait (walrus limit)."""
    bad = []
    for f in nc.m.functions:
        for bb in f.blocks:
            for inst in bb.instructions:
                si = inst.sync_info
                nw = len(si.on_wait) if si and si.on_wait else 0
                if nw > 1:
                    bad.append((inst.name, type(inst).__name__, nw,
                                [w.ant_name for w in si.on_wait]))
    return bad


def _q8(a):
    return np.asarray(a, dtype=np.float32).astype(E4M3)


def _pack_weights(conv_w, conv_b, lora_A, lora_B):
    w_eff = (conv_w.astype(np.float64) + SCALING * np.einsum(
        "or,rik->oik", lora_B.astype(np.float64),
        lora_A.astype(np.float64).reshape(RANK, CI, K))).astype(np.float32)
    W16 = _q8(16.0 * w_eff)
    w_lo = w_eff - W16.astype(np.float32) / 16.0
    energies = [(float(np.square(w_lo[:, :, k]).sum()), k) for k in range(K)]
    corr_taps = tuple(sorted(k for _, k in
                             sorted(energies, reverse=True)[:N_CORR]))
    WCOR = _q8(16.0 * w_lo)

    def pack(w8, taps, order):
        a = w8.astype(np.float32).reshape(2, 128, 2, 128, K)  # [c, m, i, p, k]
        a = a[:, :, :, :, list(taps)]                          # [c, m, i, p, j]
        a = a.transpose(order)
        return np.ascontiguousarray(
            a.reshape(128, len(taps) * 2 * 2 * 128)).astype(E4M3)

    # wts[p, c, k, i, m]; wcor[p, j, c, i, m]
    wts = pack(W16, range(K), (3, 0, 4, 2, 1))
    wcor = pack(WCOR, corr_taps, (3, 4, 0, 2, 1)) if N_CORR else None
    bias = np.ascontiguousarray(
        conv_b.astype(np.float32).reshape(2, 128).T)  # [128, 2]
    return wts, wcor, bias, corr_taps


_CACHED = {}


def kernel(x, conv_w, conv_b, lora_A, lora_B, _trace=False):
    x = np.asarray(x, dtype=np.float32)
    wts, wcor, bias, corr_taps = _pack_weights(
        np.asarray(conv_w), np.asarray(conv_b),
        np.asarray(lora_A), np.asarray(lora_B))
    zeros = np.zeros((128, 4, PAD), dtype=E4M3)

    if corr_taps not in _CACHED:
        nc = _build_nc(corr_taps)
        bad = check_waits(nc)
        assert not bad, f"sync-wait violations: {bad[:5]}"
        _CACHED[corr_taps] = nc
    nc = _CACHED[corr_taps]
    # test.py compatibility handle
    kernel.__globals__["_CACHED_NC"] = nc

    x_hi = _q8(x)
    x_lo = _q8(x - x_hi.astype(np.float32))
    # xpack[core][p, s, t]; s = hl*2 + ci_blk
    xp = np.stack([x_hi.reshape(B, 2, 128, T), x_lo.reshape(B, 2, 128, T)],
                  axis=1)                       # [B, hl, blk, p, t]
    xp = np.ascontiguousarray(xp.transpose(0, 3, 1, 2, 4)  # [B, p, hl, blk, t]
                              .reshape(B, 128, 4, T))

    in_maps = []
    for i in range(N_CORES):
        m = {"x": xp[i], "wts": wts, "bias": bias, "zeros": zeros}
        if wcor is not None:
            m["wcor"] = wcor
        in_maps.append(m)
    res = bass_utils.run_bass_kernel_spmd(
        nc, in_maps, core_ids=list(range(N_CORES)), trace=_trace)
    outs = []
    for i in range(N_CORES):
        yc = np.concatenate([np.asarray(res.results[i][f"y{s}"])
                             for s in range(NCHUNK // 2)], axis=2)
        outs.append(yc.transpose(1, 0, 2).reshape(CO, T))
    out = np.stack(outs, axis=0).astype(np.float32)
    if _trace:
        kernel._last_exec_time_ns = res.exec_time_ns
        kernel._last_results = res
    return out


_CACHED_NC = None


if __name__ == "__main__":
    nc = _build_nc((0, 2))
    bad = check_waits(nc)
    print("violations:", bad[:10])
    n_inst = sum(len(bb.instructions) for f in nc.m.functions for bb in f.blocks)
    print("instructions:", n_inst)
    from concourse.timeline_sim import TimelineSim
    dur = TimelineSim(nc, trace=False).simulate()
    print(f"TimelineSim: {dur:.0f} ns")


# revision 67
# speedup vs baseline: 1.0095x; 1.0024x over previous
"""Conv1d (K=5, pad=2) with folded LoRA on 8 Trainium2 NeuronCores.

Strategy
--------
Data-parallel: batch 8 -> 1 batch item per core. LoRA is folded into the
conv weights on the host:
    w_eff = conv_w + (alpha/rank) * einsum('or,rik->oik', lora_B, lora_A)

The device kernel runs entirely in fp8-e4m3 DoubleRow matmuls (0.5
cycles/output-column with a 256-wide contraction -- 4x the per-column fp32r
rate). Precision is recovered with a hi/lo split computed on the host:

    x_hi = e4m3(x)                  x_lo  = e4m3(x - x_hi)
    W16  = e4m3(16*w_eff)           WCOR  = e4m3(16*(w_eff - W16/16))

    psum = W16@x_hi + W16@x_lo + WCOR@x_hi        (WCOR only for the
    y    = psum/16 + bias                          N_CORR highest-energy taps)

Per psum tile [128co, 512t]: 5 hi + 5 lo + N_CORR correction DoubleRow
matmuls, each pairing the two ci-blocks in the DoubleRow slots. Eviction is
one DVE tensor_scalar affine (psum * 1/16 + bias) straight to fp16 staging;
outputs travel as fp16 and are upcast on the host. Measured end-to-end
rel-err vs the fp64 reference: 1.9e-2 @ N_CORR=2 (gate 2e-2), 1.7e-2 @ 3.

Toolchain constraint baked into the structure: every instruction may carry
at most ONE sync wait (walrus setupSyncWait limit), and Tile's wait elision
is per-proc. Hence (same architecture as the fp32r predecessor):
  - PE "observer" matmuls (tiny, scratch PSUM) absorb each x/weight DMA
    lane wait so real matmuls only wait on the DVE sem (PSUM-bank WAR).
    Observers for later-needed tensors (co1 weights, wcor, chunk-0 second
    half) are deferred to just before their first consumer so the stream
    starts as soon as the first ~1.2 MB lands.
  - All input DMAs are issued upfront on the SP HWDGE ring so the stores
    (SWDGE, one fresh DMASW lane each, at most 8) queue behind them on the
    shared DMA engines and never delay a load the PE is about to need.
  - Evictions run exclusively on DVE and wait only on PE; the fp16 staging
    tile covers the full output (no reuse -> no WAR gates); the bias lane
    is absorbed by a tiny DVE copy. The final store covers only the last
    512 columns so the post-stream tail is short.
  - A tail chain of 1-dep sync nops covers all procs so the exit drain
    carries at most one wait.

TimelineSim (the graded cost model): 91557 ns vs 177471 ns for the fp32r
predecessor. Steady-state marginal cost per 1024-column chunk is 5138 ns
vs the 5120 ns PE ideal; the remaining ~9.6 us is fixed startup (DMA
pipeline lead + first transfers, ~3.1 us), eviction+store tail (~2.3 us),
and Tile's exit drain/barrier sequence (~2.5 us).
"""
import sys
sys.path.insert(0, "/opt/trn_rl_repo")
import numpy as np
import ml_dtypes

from concourse import bass, mybir, tile
from concourse import bass_utils
from concourse.tile import add_dep_helper

E4M3 = ml_dtypes.float8_e4m3fn

# Problem constants (hardcoded per contract)
B = 8
CI = 256
CO = 256
K = 5
PAD = 2
T = 16384
RANK = 8
ALPHA = 16.0
SCALING = ALPHA / RANK
N_CORES = 8

N_CORR = 2            # correction taps (w-error fix); 2 -> rel_err ~1.9e-2

# Tiling
CHUNK = 1024          # output columns per chunk
NCHUNK = T // CHUNK   # 16
SUB = 512             # matmul free dim (one PSUM bank)
NSUB = CHUNK // SUB   # 2
XCOLS = CHUNK + 2 * PAD  # chunk + halo


def _build_nc(corr_taps, _probe_no_evict=False, _probe_no_store=False):
    f32 = mybir.dt.float32
    f16 = mybir.dt.float16
    f8 = mybir.dt.float8e4
    DR = mybir.MatmulPerfMode.DoubleRow
    n_corr = len(corr_taps)

    nc = bass.Bass(trn_type="TRN2", debug=False)
    # x slots: 0,1 = x_hi(ci blk 0/1); 2,3 = x_lo(ci blk 0/1)
    x = nc.dram_tensor("x", [128, 4, T], f8, kind="ExternalInput").ap()
    wts = nc.dram_tensor("wts", [128, K * 2 * 2 * 128], f8,
                         kind="ExternalInput").ap()
    bias = nc.dram_tensor("bias", [128, 2], f32, kind="ExternalInput").ap()
    zeros = nc.dram_tensor("zeros", [128, 4, PAD], f8, kind="ExternalInput").ap()
    if n_corr:
        wcor = nc.dram_tensor("wcor", [128, n_corr * 2 * 2 * 128], f8,
                              kind="ExternalInput").ap()
    # eight output tensors (one per SWDGE store, fresh DMASW lane each); the
    # last covers only the final 512 columns so the tail transfer is short.
    # Host concatenates along columns.
    Y_COLS = [4 * CHUNK] + [2 * CHUNK] * 5 + [CHUNK + SUB, SUB]
    ys = [nc.dram_tensor(f"y{s}", [128, 2, w], f16, kind="ExternalOutput").ap()
          for s, w in enumerate(Y_COLS)]

    NPB = 5   # psum accumulation banks
    NWARM = 0   # PE warmup matmuls: no-op under TimelineSim's wall-clock
                # p-state model; kept as a knob for real-HW experiments

    with tile.TileContext(nc) as tc:
        with tc.tile_pool(name="wp", bufs=1) as wp, \
             tc.tile_pool(name="pp", bufs=1, space="PSUM") as pp:

            # write-once observer scratch: four columns per observer matmul
            obs_ps = pp.tile([128, 96], f32, name="obs_ps", tag="obs")
            pbufs = [pp.tile([128, SUB], f32, name=f"pt{j}", tag=f"pt{j}")
                     for j in range(NPB)]
            # fresh dedicated banks for the two split tail groups
            # (first-epoch, fully owned: no cross-epoch or cross-group deps)
            sp_a = pp.tile([128, SUB // 2], f32, name="sp_a", tag="sp_a")
            sp_b = pp.tile([128, SUB // 2], f32, name="sp_b", tag="sp_b")
            # write-once DVE gate scratch for the split-tail evictions
            gs = wp.tile([128, 8], f32, name="gs")
            n_gate = [0]
            # x is fully resident: one dedicated buffer per chunk, no reuse
            xbufs = [wp.tile([128, 4, XCOLS], f8, name=f"xt{j}", tag=f"xt{j}")
                     for j in range(NCHUNK)]
            # single full-width staging tile: stores slice arbitrary ranges
            ot_all = wp.tile([128, 2, T], f16, name="ot_all")

            if NWARM:
                # PE warmup: junk tile filled by DVE at t0; matmuls on it ramp
                # the PE p-state while the input DMAs stream in.
                junk = wp.tile([128, 2, 256], f8, name="junk")
                wu_ms = nc.vector.memset(junk[:], 0.0)
                for wi in range(NWARM):
                    wm = nc.tensor.matmul(
                        pbufs[0][:, 0:256],
                        junk[:, :, 0:128], junk[:, :, 0:256],
                        start=True, stop=True, perf_mode=DR)
                    if wi == 0:
                        add_dep_helper(wm.ins, wu_ms.ins, sync=True,
                                       reason="warmup")

            wt = wp.tile([128, 2, K, 2, 128], f8, name="wt")
            wview = wts[:].rearrange("p (c k i m) -> p c k i m", c=2, k=K, i=2)
            bs = wp.tile([128, 2], f32, name="bs")

            n_obs = [0]

            def pe_observe(src_ap, dma_inst):
                """Tiny matmul whose only wait is `dma_inst`'s lane.

                Reads only within the region `dma_inst` wrote; writes its own
                never-reused obs_ps columns (no WAW chain)."""
                n = src_ap.shape[-1]
                m = min(4, n)
                oc = 4 * n_obs[0]
                n_obs[0] += 1
                mm = nc.tensor.matmul(obs_ps[0:m, oc:oc + m], src_ap[:, 0:m],
                                      src_ap[:, 0:m], start=True, stop=True)
                add_dep_helper(mm.ins, dma_inst.ins, sync=False,
                               reason="obs-order")
                return mm

            # --- all input DMAs issued upfront (SP HWDGE ring) so stores
            # queue behind them on the shared DMA engines and never delay a
            # load the PE is about to need. Ordered so the first matmul
            # group's deps (co0 weights + chunk-0 first half) land first;
            # later-needed tensors (co1 weights, wcor, bias, chunk-0 second
            # half) follow, each observed just before its first consumer.
            in_dmas = [[] for _ in range(NCHUNK)]
            # cols of chunk 0 needed by its first (ts=0) groups; chosen so
            # both DMA halves have >= 512-byte runs (single-rate DMA)
            HALF0 = SUB + PAD + PAD
            # chunk-0 head zeros via DVE memset: skips the serialized DMA
            # device so the first matmul group starts as soon as the co0
            # weights + chunk-0 first half land (~2.7us)
            z0_ms = nc.vector.memset(xbufs[0][:, :, 0:PAD], 0.0)
            d_w0 = nc.sync.dma_start(out=wt[:, 0], in_=wview[:, 0])
            in_dmas[0].append(nc.sync.dma_start(
                out=xbufs[0][:, :, PAD:HALF0],
                in_=x[:, :, 0:HALF0 - PAD]))
            d_w1 = nc.sync.dma_start(out=wt[:, 1], in_=wview[:, 1])
            if n_corr:
                wc = wp.tile([128, n_corr, 2, 2, 128], f8, name="wc")
                d_wc = nc.sync.dma_start(
                    out=wc[:],
                    in_=wcor[:].rearrange("p (j c i m) -> p j c i m",
                                          j=n_corr, c=2, i=2))
            d_b = nc.sync.dma_start(out=bs[:], in_=bias[:])
            in_dmas[0].append(nc.sync.dma_start(
                out=xbufs[0][:, :, HALF0:XCOLS],
                in_=x[:, :, HALF0 - PAD:CHUNK + PAD]))
            for c in range(1, NCHUNK):
                lo = c * CHUNK - PAD
                if c == NCHUNK - 1:
                    in_dmas[c].append(nc.sync.dma_start(
                        out=xbufs[c][:, :, 0:CHUNK + PAD],
                        in_=x[:, :, lo:T]))
                    in_dmas[c].append(nc.sync.dma_start(
                        out=xbufs[c][:, :, CHUNK + PAD:XCOLS],
                        in_=zeros[:]))
                else:
                    in_dmas[c].append(nc.sync.dma_start(
                        out=xbufs[c][:], in_=x[:, :, lo:lo + XCOLS]))

            obs_w0 = pe_observe(wt[:, 0, 0, 0], d_w0)
            # deferred observers, emitted just before their first consumer
            pend_w1 = [d_w1]
            pend_wc = [d_wc] if n_corr else []
            # DVE observes the bias lane via a write-once copy
            bscratch = wp.tile([128, 2], f32, name="bscratch")
            obs_b = nc.vector.tensor_copy(bscratch[:], bs[:])

            out_dmas = []     # store DMAs
            last_mm = None
            last_evict = None
            pi = 0            # psum bank rotation

            def emit_store(s):
                # SWDGE store (fresh DMASW lane each): carries only its
                # staging-ready (DVE evict) wait.
                col0 = sum(Y_COLS[:s])
                out_dmas.append(nc.gpsimd.dma_start(
                    out=ys[s][:], in_=ot_all[:, :, col0:col0 + Y_COLS[s]]))

            first_evict = [True]

            def do_group(c, ts, co, seg, pt_override=None, gate_evict=False):
                """One psum accumulation group + eviction. seg: (lo, n) cols
                within the subtile; pt_override places the group in a
                dedicated psum tile at that offset (split tail groups).
                gate_evict pre-lifts the group's PE-stop wait onto DVE via a
                write-once memset so the evict's only wait is its same-bucket
                WAW on the previous split evict (walrus one-wait limit)."""
                nonlocal last_mm, last_evict, pi
                slo, sn = seg
                xt = xbufs[c]
                if pt_override is not None:
                    pt = pt_override
                    plo = 0          # dedicated tile, exactly sn wide
                else:
                    pt = pbufs[pi % NPB]
                    pi += 1
                    plo = slo
                mm_specs = (
                    [(wt[:, co, k], 0, k) for k in range(K)] +
                    [(wt[:, co, k], 2, k) for k in range(K)] +
                    [(wc[:, j, co], 0, k) for j, k in enumerate(corr_taps)])
                for n_i, (st, sl, k) in enumerate(mm_specs):
                    if n_i == 10 and pend_wc:
                        observers.append(pe_observe(wc[:, 0, 0, 0],
                                                    pend_wc.pop()))
                    off = ts * SUB + slo + k
                    mm = nc.tensor.matmul(
                        pt[:, plo:plo + sn],
                        st,
                        xt[:, sl:sl + 2, off:off + sn],
                        start=(n_i == 0),
                        stop=(n_i == len(mm_specs) - 1),
                        perf_mode=DR,
                    )
                    while observers:
                        add_dep_helper(mm.ins, observers.pop().ins,
                                       sync=False, reason="order-after-obs")
                    last_mm = mm
                if _probe_no_evict:
                    return
                if gate_evict:
                    gc = n_gate[0]
                    n_gate[0] += 1
                    ms = nc.vector.memset(gs[:, gc:gc + 1], 0.0)
                    add_dep_helper(ms.ins, last_mm.ins, sync=True,
                                   reason="evict-gate")
                off = c * CHUNK + ts * SUB + slo
                ev = nc.vector.tensor_scalar(
                    out=ot_all[:, co, off:off + sn],
                    in0=pt[:, plo:plo + sn],
                    scalar1=1.0 / 16.0,
                    scalar2=bs[:, co:co + 1],
                    op0=mybir.AluOpType.mult,
                    op1=mybir.AluOpType.add,
                )
                if first_evict[0]:
                    add_dep_helper(ev.ins, obs_b.ins, sync=False,
                                   reason="order-after-gate")
                    first_evict[0] = False
                last_evict = ev

            observers = []
            for c in range(NCHUNK):
                xt = xbufs[c]

                # PE observes this chunk's x lanes; each observer reads only
                # within its DMA's region. Chunk 0's second-half observer is
                # deferred until its ts=1 groups so ts=0 can start early.
                late_x = []
                for i, d in enumerate(in_dmas[c]):
                    if c == 0:
                        src_ap = [xt[:, 0, PAD:PAD + 4],
                                  xt[:, 0, HALF0:HALF0 + 4]][i]
                    elif c == NCHUNK - 1:
                        src_ap = [xt[:, 0, 0:4],
                                  xt[:, 0, CHUNK + PAD:XCOLS]][i]
                    else:
                        src_ap = xt[:, 0, 0:4]
                    if c == 0 and i == 1:
                        late_x.append((src_ap, d))
                    else:
                        observers.append(pe_observe(src_ap, d))
                if c == 0:
                    observers.append(obs_w0)

                for ts in range(NSUB):
                    if late_x and ts == 1:
                        observers.extend(pe_observe(s, d) for s, d in late_x)
                        late_x = []
                    for co in range(2):
                        if pend_w1 and co == 1:
                            observers.append(pe_observe(wt[:, 1, 0, 0],
                                                        pend_w1.pop()))
                        if c == NCHUNK - 1 and ts == NSUB - 1 and co == 1:
                            # split tail: 2x256-col groups on dedicated
                            # banks; the first eviction overlaps the second
                            # group's matmuls, shrinking the final
                            # evict->store critical path from 658 to ~391 ns
                            do_group(c, ts, co, (0, SUB // 2),
                                     pt_override=sp_a)
                            do_group(c, ts, co, (SUB // 2, SUB // 2),
                                     pt_override=sp_b, gate_evict=True)
                        else:
                            do_group(c, ts, co, (0, SUB))
                    if c == NCHUNK - 1 and ts == 0 and not _probe_no_evict \
                            and not _probe_no_store:
                        emit_store(6)   # chunk 14 + first half of chunk 15
                if _probe_no_evict or _probe_no_store:
                    continue
                if c in (3, 5, 7, 9, 11, 13):
                    emit_store((3, 5, 7, 9, 11, 13).index(c))

            if not _probe_no_evict and not _probe_no_store:
                emit_store(7)           # final 512 columns

            # Tail flush: cover every proc with 1-dep sync nops so the final
            # drain carries at most one wait.
            tail_deps = [d for ds in in_dmas[-8:] for d in ds] + out_dmas + \
                [last_mm, last_evict]
            for dep in tail_deps:
                if dep is None:
                    continue
                nop = nc.sync.nop()
                add_dep_helper(nop.ins, dep.ins, sync=True, reason="tailflush")

    return nc


def check_waits(nc):
    """Return instructions carrying more than one sync wait (walrus limit)."""
    bad = []
    for f in nc.m.functions:
        for bb in f.blocks:
            for inst in bb.instructions:
                si = inst.sync_info
                nw = len(si.on_wait) if si and si.on_wait else 0
                if nw > 1:
                    bad.append((inst.name, type(inst).__name__, nw,
                                [w.ant_name for w in si.on_wait]))
    return bad


def _q8(a):
    return np.asarray(a, dtype=np.float32).astype(E4M3)


def _pack_weights(conv_w, conv_b, lora_A, lora_B):
    w_eff = (conv_w.astype(np.float64) + SCALING * np.einsum(
        "or,rik->oik", lora_B.astype(np.float64),
        lora_A.astype(np.float64).reshape(RANK, CI, K))).astype(np.float32)
    W16 = _q8(16.0 * w_eff)
    w_lo = w_eff - W16.astype(np.float32) / 16.0
    energies = [(float(np.square(w_lo[:, :, k]).sum()), k) for k in range(K)]
    corr_taps = tuple(sorted(k for _, k in
                             sorted(energies, reverse=True)[:N_CORR]))
    WCOR = _q8(16.0 * w_lo)

    def pack(w8, taps, order):
        a = w8.astype(np.float32).reshape(2, 128, 2, 128, K)  # [c, m, i, p, k]
        a = a[:, :, :, :, list(taps)]                          # [c, m, i, p, j]
        a = a.transpose(order)
        return np.ascontiguousarray(
            a.reshape(128, len(taps) * 2 * 2 * 128)).astype(E4M3)

    # wts[p, c, k, i, m]; wcor[p, j, c, i, m]
    wts = pack(W16, range(K), (3, 0, 4, 2, 1))
    wcor = pack(WCOR, corr_taps, (3, 4, 0, 2, 1)) if N_CORR else None
    bias = np.ascontiguousarray(
        conv_b.astype(np.float32).reshape(2, 128).T)  # [128, 2]
    return wts, wcor, bias, corr_taps


_CACHED = {}


def kernel(x, conv_w, conv_b, lora_A, lora_B, _trace=False):
    x = np.asarray(x, dtype=np.float32)
    wts, wcor, bias, corr_taps = _pack_weights(
        np.asarray(conv_w), np.asarray(conv_b),
        np.asarray(lora_A), np.asarray(lora_B))
    zeros = np.zeros((128, 4, PAD), dtype=E4M3)

    if corr_taps not in _CACHED:
        nc = _build_nc(corr_taps)
        bad = check_waits(nc)
        assert not bad, f"sync-wait violations: {bad[:5]}"
        _CACHED[corr_taps] = nc
    nc = _CACHED[corr_taps]
    # test.py compatibility handle
    kernel.__globals__["_CACHED_NC"] = nc

    x_hi = _q8(x)
    x_lo = _q8(x - x_hi.astype(np.float32))
    # xpack[core][p, s, t]; s = hl*2 + ci_blk
    xp = np.stack([x_hi.reshape(B, 2, 128, T), x_lo.reshape(B, 2, 128, T)],
                  axis=1)                       # [B, hl, blk, p, t]
    xp = np.ascontiguousarray(xp.transpose(0, 3, 1, 2, 4)  # [B, p, hl, blk, t]
                              .reshape(B, 128, 4, T))

    in_maps = []
    for i in range(N_CORES):
        m = {"x": xp[i], "wts": wts, "bias": bias, "zeros": zeros}
        if wcor is not None:
            m["wcor"] = wcor
        in_maps.append(m)
    res = bass_utils.run_bass_kernel_spmd(
        nc, in_maps, core_ids=list(range(N_CORES)), trace=_trace)
    outs = []
    for i in range(N_CORES):
        yc = np.concatenate([np.asarray(res.results[i][f"y{s}"])
                             for s in range(NCHUNK // 2)], axis=2)
        outs.append(yc.transpose(1, 0, 2).reshape(CO, T))
    out = np.stack(outs, axis=0).astype(np.float32)
    if _trace:
        kernel._last_exec_time_ns = res.exec_time_ns
        kernel._last_results = res
    return out


_CACHED_NC = None


if __name__ == "__main__":
    nc = _build_nc((0, 2))
    bad = check_waits(nc)
    print("violations:", bad[:10])
    n_inst = sum(len(bb.instructions) for f in nc.m.functions for bb in f.blocks)
    print("instructions:", n_inst)
    from concourse.timeline_sim import TimelineSim
    dur = TimelineSim(nc, trace=False).simulate()
    print(f"TimelineSim: {dur:.0f} ns")


# revision 76
# speedup vs baseline: 1.0133x; 1.0037x over previous
"""Conv1d (K=5, pad=2) with folded LoRA on 8 Trainium2 NeuronCores.

Strategy
--------
Data-parallel: batch 8 -> 1 batch item per core. LoRA is folded into the
conv weights on the host:
    w_eff = conv_w + (alpha/rank) * einsum('or,rik->oik', lora_B, lora_A)

The device kernel runs entirely in fp8-e4m3 DoubleRow matmuls (0.5
cycles/output-column with a 256-wide contraction -- 4x the per-column fp32r
rate). Precision is recovered with a hi/lo split computed on the host:

    x_hi = e4m3(x)                  x_lo  = e4m3(x - x_hi)
    W16  = e4m3(16*w_eff)           WCOR  = e4m3(16*(w_eff - W16/16))

    psum = W16@x_hi + W16@x_lo + WCOR@x_hi        (WCOR only for the
    y    = psum/16 + bias                          N_CORR highest-energy taps)

Per psum tile [128co, 512t]: 5 hi + 5 lo + N_CORR correction DoubleRow
matmuls, each pairing the two ci-blocks in the DoubleRow slots. Eviction is
one DVE tensor_scalar affine (psum * 1/16 + bias) straight to fp16 staging;
outputs travel as fp16 and are upcast on the host. Measured end-to-end
rel-err vs the fp64 reference: 1.9e-2 @ N_CORR=2 (gate 2e-2), 1.7e-2 @ 3.

Toolchain constraint baked into the structure: every instruction may carry
at most ONE sync wait (walrus setupSyncWait limit), and Tile's wait elision
is per-proc. Hence (same architecture as the fp32r predecessor):
  - PE "observer" matmuls (tiny, scratch PSUM) absorb each x/weight DMA
    lane wait so real matmuls only wait on the DVE sem (PSUM-bank WAR).
    Observers for later-needed tensors (co1 weights, wcor, chunk-0 second
    half) are deferred to just before their first consumer so the stream
    starts as soon as the first ~1.2 MB lands.
  - All input DMAs are issued upfront so the stores queue behind them on
    the shared DMA engines and never delay a load the PE is about to need.
    Only the two startup-critical loads ride the SP HWDGE ring; the rest go
    SWDGE, leaving six fresh HWDGE lanes for the six stores (which then
    carry only their staging-ready wait and take the cheaper HWDGE tail).
  - Evictions run exclusively on DVE and wait only on PE; the fp16 staging
    tile covers the full output (no reuse -> no WAR gates); the bias lane
    is absorbed by a tiny DVE copy. The final store covers only the last
    512 columns so the post-stream tail is short.
  - A tail chain of 1-dep sync nops covers all procs so the exit drain
    carries at most one wait.

TimelineSim (the graded cost model): 91215 ns vs 177471 ns for the fp32r
predecessor. Steady-state marginal cost per 1024-column chunk is ~5138 ns
vs the 5120 ns PE ideal; the remaining ~9.3 us is fixed startup (DMA
pipeline lead + first transfers, ~3.1 us), eviction+store tail (~1.8 us),
and Tile's exit drain/barrier sequence (~2.5 us).
"""
import sys
sys.path.insert(0, "/opt/trn_rl_repo")
import numpy as np
import ml_dtypes

from concourse import bass, mybir, tile
from concourse import bass_utils
from concourse.tile import add_dep_helper

E4M3 = ml_dtypes.float8_e4m3fn

# Problem constants (hardcoded per contract)
B = 8
CI = 256
CO = 256
K = 5
PAD = 2
T = 16384
RANK = 8
ALPHA = 16.0
SCALING = ALPHA / RANK
N_CORES = 8

N_CORR = 2            # correction taps (w-error fix); 2 -> rel_err ~1.9e-2

# Tiling
CHUNK = 1024          # output columns per chunk
NCHUNK = T // CHUNK   # 16
SUB = 512             # matmul free dim (one PSUM bank)
NSUB = CHUNK // SUB   # 2
XCOLS = CHUNK + 2 * PAD  # chunk + halo


def _build_nc(corr_taps, _probe_no_evict=False, _probe_no_store=False):
    f32 = mybir.dt.float32
    f16 = mybir.dt.float16
    f8 = mybir.dt.float8e4
    DR = mybir.MatmulPerfMode.DoubleRow
    n_corr = len(corr_taps)

    nc = bass.Bass(trn_type="TRN2", debug=False)
    # x slots: 0,1 = x_hi(ci blk 0/1); 2,3 = x_lo(ci blk 0/1)
    x = nc.dram_tensor("x", [128, 4, T], f8, kind="ExternalInput").ap()
    wts = nc.dram_tensor("wts", [128, K * 2 * 2 * 128], f8,
                         kind="ExternalInput").ap()
    bias = nc.dram_tensor("bias", [128, 2], f32, kind="ExternalInput").ap()
    zeros = nc.dram_tensor("zeros", [128, 4, PAD], f8, kind="ExternalInput").ap()
    if n_corr:
        wcor = nc.dram_tensor("wcor", [128, n_corr * 2 * 2 * 128], f8,
                              kind="ExternalInput").ap()
    # six output tensors (one per SP-HWDGE store, fresh lane each); the last
    # covers only the final 512 columns so the tail transfer is short. Host
    # concatenates along columns.
    Y_COLS = [4 * CHUNK] * 3 + [2 * CHUNK, CHUNK + SUB, SUB]
    ys = [nc.dram_tensor(f"y{s}", [128, 2, w], f16, kind="ExternalOutput").ap()
          for s, w in enumerate(Y_COLS)]

    NPB = 5   # psum accumulation banks
    NWARM = 0   # PE warmup matmuls: no-op under TimelineSim's wall-clock
                # p-state model; kept as a knob for real-HW experiments

    with tile.TileContext(nc) as tc:
        with tc.tile_pool(name="wp", bufs=1) as wp, \
             tc.tile_pool(name="pp", bufs=1, space="PSUM") as pp:

            # write-once observer scratch: four columns per observer matmul
            obs_ps = pp.tile([128, 96], f32, name="obs_ps", tag="obs")
            pbufs = [pp.tile([128, SUB], f32, name=f"pt{j}", tag=f"pt{j}")
                     for j in range(NPB)]
            # fresh dedicated banks for the two split tail groups
            # (first-epoch, fully owned: no cross-epoch or cross-group deps)
            sp_a = pp.tile([128, SUB // 2], f32, name="sp_a", tag="sp_a")
            sp_b = pp.tile([128, SUB // 2], f32, name="sp_b", tag="sp_b")
            # write-once DVE gate scratch for the split-tail evictions
            gs = wp.tile([128, 8], f32, name="gs")
            n_gate = [0]
            # x is fully resident: one dedicated buffer per chunk, no reuse
            xbufs = [wp.tile([128, 4, XCOLS], f8, name=f"xt{j}", tag=f"xt{j}")
                     for j in range(NCHUNK)]
            # single full-width staging tile: stores slice arbitrary ranges
            ot_all = wp.tile([128, 2, T], f16, name="ot_all")

            if NWARM:
                # PE warmup: junk tile filled by DVE at t0; matmuls on it ramp
                # the PE p-state while the input DMAs stream in.
                junk = wp.tile([128, 2, 256], f8, name="junk")
                wu_ms = nc.vector.memset(junk[:], 0.0)
                for wi in range(NWARM):
                    wm = nc.tensor.matmul(
                        pbufs[0][:, 0:256],
                        junk[:, :, 0:128], junk[:, :, 0:256],
                        start=True, stop=True, perf_mode=DR)
                    if wi == 0:
                        add_dep_helper(wm.ins, wu_ms.ins, sync=True,
                                       reason="warmup")

            wt = wp.tile([128, 2, K, 2, 128], f8, name="wt")
            wview = wts[:].rearrange("p (c k i m) -> p c k i m", c=2, k=K, i=2)
            bs = wp.tile([128, 2], f32, name="bs")

            n_obs = [0]

            def pe_observe(src_ap, dma_inst):
                """Tiny matmul whose only wait is `dma_inst`'s lane.

                Reads only within the region `dma_inst` wrote; writes its own
                never-reused obs_ps columns (no WAW chain)."""
                n = src_ap.shape[-1]
                m = min(4, n)
                oc = 4 * n_obs[0]
                n_obs[0] += 1
                mm = nc.tensor.matmul(obs_ps[0:m, oc:oc + m], src_ap[:, 0:m],
                                      src_ap[:, 0:m], start=True, stop=True)
                add_dep_helper(mm.ins, dma_inst.ins, sync=False,
                               reason="obs-order")
                return mm

            # --- all input DMAs issued upfront. Only the two startup-
            # critical loads (co0 weights + chunk-0 first half) ride the SP
            # HWDGE ring; everything else goes SWDGE so six of the eight
            # HWDGE lanes stay fresh for the stores (which then carry only
            # their staging-ready wait and take the cheaper HWDGE tail
            # path). SWDGE order puts the soonest-needed tensors (wcor, co1
            # weights, chunk-0 second half, bias) ahead of the bulk chunks.
            in_dmas = [[] for _ in range(NCHUNK)]
            # cols of chunk 0 needed by its first (ts=0) groups; chosen so
            # both DMA halves have >= 512-byte runs (single-rate DMA)
            HALF0 = SUB + PAD + PAD
            # chunk-0 head zeros via DVE memset: skips the serialized DMA
            # device so the first matmul group starts as soon as the co0
            # weights + chunk-0 first half land (~2.7us)
            z0_ms = nc.vector.memset(xbufs[0][:, :, 0:PAD], 0.0)
            d_w0 = nc.sync.dma_start(out=wt[:, 0], in_=wview[:, 0])
            in_dmas[0].append(nc.sync.dma_start(
                out=xbufs[0][:, :, PAD:HALF0],
                in_=x[:, :, 0:HALF0 - PAD]))
            if n_corr:
                wc = wp.tile([128, n_corr, 2, 2, 128], f8, name="wc")
                d_wc = nc.gpsimd.dma_start(
                    out=wc[:],
                    in_=wcor[:].rearrange("p (j c i m) -> p j c i m",
                                          j=n_corr, c=2, i=2))
            d_w1 = nc.gpsimd.dma_start(out=wt[:, 1], in_=wview[:, 1])
            in_dmas[0].append(nc.gpsimd.dma_start(
                out=xbufs[0][:, :, HALF0:XCOLS],
                in_=x[:, :, HALF0 - PAD:CHUNK + PAD]))
            d_b = nc.gpsimd.dma_start(out=bs[:], in_=bias[:])
            for c in range(1, NCHUNK):
                lo = c * CHUNK - PAD
                if c == NCHUNK - 1:
                    in_dmas[c].append(nc.gpsimd.dma_start(
                        out=xbufs[c][:, :, 0:CHUNK + PAD],
                        in_=x[:, :, lo:T]))
                    in_dmas[c].append(nc.gpsimd.dma_start(
                        out=xbufs[c][:, :, CHUNK + PAD:XCOLS],
                        in_=zeros[:]))
                else:
                    in_dmas[c].append(nc.gpsimd.dma_start(
                        out=xbufs[c][:], in_=x[:, :, lo:lo + XCOLS]))

            obs_w0 = pe_observe(wt[:, 0, 0, 0], d_w0)
            # deferred observers, emitted just before their first consumer
            pend_w1 = [d_w1]
            pend_wc = [d_wc] if n_corr else []
            # DVE observes the bias lane via a write-once copy
            bscratch = wp.tile([128, 2], f32, name="bscratch")
            obs_b = nc.vector.tensor_copy(bscratch[:], bs[:])

            out_dmas = []     # store DMAs
            last_mm = None
            last_evict = None
            pi = 0            # psum bank rotation

            def emit_store(s):
                # SP-HWDGE store on a fresh lane (only w0 and x0a ride this
                # ring ahead of them): carries only its staging-ready (DVE
                # evict) wait, and takes the cheaper HWDGE tail path.
                col0 = sum(Y_COLS[:s])
                out_dmas.append(nc.sync.dma_start(
                    out=ys[s][:], in_=ot_all[:, :, col0:col0 + Y_COLS[s]]))

            first_evict = [True]

            def do_group(c, ts, co, seg, pt_override=None, gate_evict=False):
                """One psum accumulation group + eviction. seg: (lo, n) cols
                within the subtile; pt_override places the group in a
                dedicated psum tile at that offset (split tail groups).
                gate_evict pre-lifts the group's PE-stop wait onto DVE via a
                write-once memset so the evict's only wait is its same-bucket
                WAW on the previous split evict (walrus one-wait limit)."""
                nonlocal last_mm, last_evict, pi
                slo, sn = seg
                xt = xbufs[c]
                if pt_override is not None:
                    pt = pt_override
                    plo = 0          # dedicated tile, exactly sn wide
                else:
                    pt = pbufs[pi % NPB]
                    pi += 1
                    plo = slo
                mm_specs = (
                    [(wt[:, co, k], 0, k) for k in range(K)] +
                    [(wt[:, co, k], 2, k) for k in range(K)] +
                    [(wc[:, j, co], 0, k) for j, k in enumerate(corr_taps)])
                for n_i, (st, sl, k) in enumerate(mm_specs):
                    if n_i == 10 and pend_wc:
                        observers.append(pe_observe(wc[:, 0, 0, 0],
                                                    pend_wc.pop()))
                    off = ts * SUB + slo + k
                    mm = nc.tensor.matmul(
                        pt[:, plo:plo + sn],
                        st,
                        xt[:, sl:sl + 2, off:off + sn],
                        start=(n_i == 0),
                        stop=(n_i == len(mm_specs) - 1),
                        perf_mode=DR,
                    )
                    while observers:
                        add_dep_helper(mm.ins, observers.pop().ins,
                                       sync=False, reason="order-after-obs")
                    last_mm = mm
                if _probe_no_evict:
                    return
                if gate_evict:
                    gc = n_gate[0]
                    n_gate[0] += 1
                    ms = nc.vector.memset(gs[:, gc:gc + 1], 0.0)
                    add_dep_helper(ms.ins, last_mm.ins, sync=True,
                                   reason="evict-gate")
                off = c * CHUNK + ts * SUB + slo
                ev = nc.vector.tensor_scalar(
                    out=ot_all[:, co, off:off + sn],
                    in0=pt[:, plo:plo + sn],
                    scalar1=1.0 / 16.0,
                    scalar2=bs[:, co:co + 1],
                    op0=mybir.AluOpType.mult,
                    op1=mybir.AluOpType.add,
                )
                if first_evict[0]:
                    add_dep_helper(ev.ins, obs_b.ins, sync=False,
                                   reason="order-after-gate")
                    first_evict[0] = False
                last_evict = ev

            observers = []
            for c in range(NCHUNK):
                xt = xbufs[c]

                # PE observes this chunk's x lanes; each observer reads only
                # within its DMA's region. Chunk 0's second-half observer is
                # deferred until its ts=1 groups so ts=0 can start early.
                late_x = []
                for i, d in enumerate(in_dmas[c]):
                    if c == 0:
                        src_ap = [xt[:, 0, PAD:PAD + 4],
                                  xt[:, 0, HALF0:HALF0 + 4]][i]
                    elif c == NCHUNK - 1:
                        src_ap = [xt[:, 0, 0:4],
                                  xt[:, 0, CHUNK + PAD:XCOLS]][i]
                    else:
                        src_ap = xt[:, 0, 0:4]
                    if c == 0 and i == 1:
                        late_x.append((src_ap, d))
                    else:
                        observers.append(pe_observe(src_ap, d))
                if c == 0:
                    observers.append(obs_w0)

                for ts in range(NSUB):
                    if late_x and ts == 1:
                        observers.extend(pe_observe(s, d) for s, d in late_x)
                        late_x = []
                    for co in range(2):
                        if pend_w1 and co == 1:
                            observers.append(pe_observe(wt[:, 1, 0, 0],
                                                        pend_w1.pop()))
                        if c == NCHUNK - 1 and ts == NSUB - 1 and co == 1:
                            # split tail: 2x256-col groups on dedicated
                            # banks; the first eviction overlaps the second
                            # group's matmuls, shrinking the final
                            # evict->store critical path from 658 to ~391 ns
                            do_group(c, ts, co, (0, SUB // 2),
                                     pt_override=sp_a)
                            do_group(c, ts, co, (SUB // 2, SUB // 2),
                                     pt_override=sp_b, gate_evict=True)
                        else:
                            do_group(c, ts, co, (0, SUB))
                    if c == NCHUNK - 1 and ts == 0 and not _probe_no_evict \
                            and not _probe_no_store:
                        emit_store(4)   # chunk 14 + first half of chunk 15
                if _probe_no_evict or _probe_no_store:
                    continue
                if c in (3, 7, 11, 13):
                    emit_store((3, 7, 11, 13).index(c))

            if not _probe_no_evict and not _probe_no_store:
                emit_store(5)           # final 512 columns

            # Tail flush: cover every proc with 1-dep sync nops so the final
            # drain carries at most one wait.
            tail_deps = [d_w0, in_dmas[0][0]] + \
                [d for ds in in_dmas[-8:] for d in ds] + out_dmas + \
                [last_mm, last_evict]
            for dep in tail_deps:
                if dep is None:
                    continue
                nop = nc.sync.nop()
                add_dep_helper(nop.ins, dep.ins, sync=True, reason="tailflush")

    return nc


def check_waits(nc):
    """Return instructions carrying more than one sync wait (walrus limit)."""
    bad = []
    for f in nc.m.functions:
        for bb in f.blocks:
            for inst in bb.instructions:
                si = inst.sync_info
                nw = len(si.on_wait) if si and si.on_wait else 0
                if nw > 1:
                    bad.append((inst.name, type(inst).__name__, nw,
                                [w.ant_name for w in si.on_wait]))
    return bad


def _q8(a):
    return np.asarray(a, dtype=np.float32).astype(E4M3)


def _pack_weights(conv_w, conv_b, lora_A, lora_B):
    w_eff = (conv_w.astype(np.float64) + SCALING * np.einsum(
        "or,rik->oik", lora_B.astype(np.float64),
        lora_A.astype(np.float64).reshape(RANK, CI, K))).astype(np.float32)
    W16 = _q8(16.0 * w_eff)
    w_lo = w_eff - W16.astype(np.float32) / 16.0
    energies = [(float(np.square(w_lo[:, :, k]).sum()), k) for k in range(K)]
    corr_taps = tuple(sorted(k for _, k in
                             sorted(energies, reverse=True)[:N_CORR]))
    WCOR = _q8(16.0 * w_lo)

    def pack(w8, taps, order):
        a = w8.astype(np.float32).reshape(2, 128, 2, 128, K)  # [c, m, i, p, k]
        a = a[:, :, :, :, list(taps)]                          # [c, m, i, p, j]
        a = a.transpose(order)
        return np.ascontiguousarray(
            a.reshape(128, len(taps) * 2 * 2 * 128)).astype(E4M3)

    # wts[p, c, k, i, m]; wcor[p, j, c, i, m]
    wts = pack(W16, range(K), (3, 0, 4, 2, 1))
    wcor = pack(WCOR, corr_taps, (3, 4, 0, 2, 1)) if N_CORR else None
    bias = np.ascontiguousarray(
        conv_b.astype(np.float32).reshape(2, 128).T)  # [128, 2]
    return wts, wcor, bias, corr_taps


_CACHED = {}


def kernel(x, conv_w, conv_b, lora_A, lora_B, _trace=False):
    x = np.asarray(x, dtype=np.float32)
    wts, wcor, bias, corr_taps = _pack_weights(
        np.asarray(conv_w), np.asarray(conv_b),
        np.asarray(lora_A), np.asarray(lora_B))
    zeros = np.zeros((128, 4, PAD), dtype=E4M3)

    if corr_taps not in _CACHED:
        nc = _build_nc(corr_taps)
        bad = check_waits(nc)
        assert not bad, f"sync-wait violations: {bad[:5]}"
        _CACHED[corr_taps] = nc
    nc = _CACHED[corr_taps]
    # test.py compatibility handle
    kernel.__globals__["_CACHED_NC"] = nc

    x_hi = _q8(x)
    x_lo = _q8(x - x_hi.astype(np.float32))
    # xpack[core][p, s, t]; s = hl*2 + ci_blk
    xp = np.stack([x_hi.reshape(B, 2, 128, T), x_lo.reshape(B, 2, 128, T)],
                  axis=1)                       # [B, hl, blk, p, t]
    xp = np.ascontiguousarray(xp.transpose(0, 3, 1, 2, 4)  # [B, p, hl, blk, t]
                              .reshape(B, 128, 4, T))

    in_maps = []
    for i in range(N_CORES):
        m = {"x": xp[i], "wts": wts, "bias": bias, "zeros": zeros}
        if wcor is not None:
            m["wcor"] = wcor
        in_maps.append(m)
    res = bass_utils.run_bass_kernel_spmd(
        nc, in_maps, core_ids=list(range(N_CORES)), trace=_trace)
    outs = []
    for i in range(N_CORES):
        yc = np.concatenate([np.asarray(res.results[i][f"y{s}"])
                             for s in range(6)], axis=2)
        outs.append(yc.transpose(1, 0, 2).reshape(CO, T))
    out = np.stack(outs, axis=0).astype(np.float32)
    if _trace:
        kernel._last_exec_time_ns = res.exec_time_ns
        kernel._last_results = res
    return out


_CACHED_NC = None


if __name__ == "__main__":
    nc = _build_nc((0, 2))
    bad = check_waits(nc)
    print("violations:", bad[:10])
    n_inst = sum(len(bb.instructions) for f in nc.m.functions for bb in f.blocks)
    print("instructions:", n_inst)
    from concourse.timeline_sim import TimelineSim
    dur = TimelineSim(nc, trace=False).simulate()
    print(f"TimelineSim: {dur:.0f} ns")
